# revision 20
# baseline (speedup 1.0000x reference)
"""Bahdanau-style attention kernel for Trainium2, data-parallel over batch B on 8 NeuronCores.

Reference computation (per batch b):
  attn1[p,a] = sum_c keys[p,b,c] * Wa_w[a,c] + Wa_b[a]
  attn2[a]   = sum_h queries[0,b,h] * Ua_w[a,h] + Ua_b[a]
  scores[p]  = sum_a tanh(attn1[p,a] + attn2[a]) * va_w[0,a] + va_b[0]
  weights    = softmax(scores over p)
  context[c] = sum_p weights[p] * keys[p,b,c]

Scores are tiny (|s| < ~4), so softmax is computed unnormalized: exp(s) is
accumulated into Z and context_unnorm in a single pass over keys, then
normalized at the end.

The PE contracts over the partition axis, so attn1 (contraction over c) needs
keys with c on partitions while the context accumulation (contraction over p)
needs p on partitions.  Both layouts are prepared host-side: keysT in fp8
(feeds only the scores path, where quantization error averages out over the
C=512 contraction) and keysN in bf16 (feeds the context accumulation).  The
device reads 1.5 MB per 128-row tile and does no transposes or casts of keys.
attn1 runs as fp8 DoubleRow matmuls (2 MACs/cell/cycle).  The small weight
operands (WaT, UaT, qT) are also laid out host-side.
"""

import sys

sys.path.insert(0, "/opt/trn_rl_repo")

import ml_dtypes
import numpy as np

from concourse import bacc, bass, mybir, tile
from concourse import bass_utils

P, B, C, H, A = 4096, 64, 512, 512, 256
NCORES = 8
BL = B // NCORES  # 8 batches per core
PT = 128  # rows per p-tile
NT = P // PT  # 32 p-tiles
NC_CH = C // 128  # 4 contraction chunks of 128
NH_CH = H // 128
F32 = mybir.dt.float32
BF16 = mybir.dt.bfloat16
FP8 = mybir.dt.float8e4
AF = mybir.ActivationFunctionType
ALU = mybir.AluOpType
PM = mybir.MatmulPerfMode


def build_nc():
    nc = bacc.Bacc("TRN2", target_bir_lowering=False, debug=False)

    # host-prepared key layouts
    # keysN[t, p, b, c]      = keys[t*128+p, b, c]            bf16 (p on partitions)
    # keysT[t, cc, b, mc, p] = keys[t*128+p, b, mc*128+cc]    fp8  (c-chunk on partitions)
    keysN_d = nc.dram_tensor("keysN", [NT, PT, BL, C], BF16, kind="ExternalInput")
    keysT_d = nc.dram_tensor(
        "keysT", [NT, 128, BL, NC_CH, PT], FP8, kind="ExternalInput"
    )
    # host-prepared weight layouts
    # waT[cc, mc, a] = Wa_w[a, mc*128+cc] fp8 ; uaT likewise bf16 ; qT[hh, hc, b] bf16
    waT_d = nc.dram_tensor("waT_h", [128, NC_CH, A], FP8, kind="ExternalInput")
    uaT_d = nc.dram_tensor("uaT_h", [128, NH_CH, A], BF16, kind="ExternalInput")
    qT_d = nc.dram_tensor("qT_h", [128, NH_CH, BL], BF16, kind="ExternalInput")
    wab_d = nc.dram_tensor("Wa_b", [A], F32, kind="ExternalInput")
    uab_d = nc.dram_tensor("Ua_b", [A], F32, kind="ExternalInput")
    vaw_d = nc.dram_tensor("va_w", [1, A], F32, kind="ExternalInput")
    vab_d = nc.dram_tensor("va_b", [1], F32, kind="ExternalInput")
    ctx_d = nc.dram_tensor("ctx_out", [BL, C], F32, kind="ExternalOutput")
    w_d = nc.dram_tensor("w_out", [P, BL], F32, kind="ExternalOutput")

    with tile.TileContext(nc) as tc:
        with (
            tc.tile_pool(name="consts", bufs=1) as consts,
            tc.tile_pool(name="setup", bufs=2) as setup,
            tc.tile_pool(name="persist", bufs=1) as persist,
            tc.tile_pool(name="keysN", bufs=5) as keysN_pool,
            tc.tile_pool(name="keysT", bufs=5) as keysT_pool,
            tc.tile_pool(name="tanh", bufs=4) as tanh_pool,
            tc.tile_pool(name="junk", bufs=4) as junk_pool,
            tc.tile_pool(name="small", bufs=6) as small_pool,
            tc.tile_pool(name="psum_a2", bufs=1, space="PSUM") as psum_a2_pool,
            tc.tile_pool(name="psum_mm1", bufs=2, space="PSUM") as psum_mm1_pool,
            tc.tile_pool(name="psum_acc", bufs=1, space="PSUM") as psum_acc_pool,
        ):
            # ---------------- constants / small weights ----------------
            waT = consts.tile([128, NC_CH, A], FP8)
            nc.sync.dma_start(waT[:], waT_d[:])
            uaT = consts.tile([128, NH_CH, A], BF16)
            nc.sync.dma_start(uaT[:], uaT_d[:])
            qT = consts.tile([128, NH_CH, BL], BF16)
            nc.sync.dma_start(qT[:], qT_d[:])

            ones_col_f32 = consts.tile([128, 1], F32)
            nc.vector.memset(ones_col_f32[:], 1.0)
            # padded-ones stationary for the attn2 add: row 0 = 1, rows 1.. = 0
            a2pad = consts.tile([128, PT], BF16)
            nc.vector.memset(a2pad[:], 0.0)
            nc.vector.memset(a2pad[0:1, :], 1.0)

            # va broadcast to all partitions, repeated 4x (for the scores reduction)
            va_f32 = setup.tile([1, A], F32)
            nc.sync.dma_start(va_f32[:], vaw_d[:])
            va_bf_row = setup.tile([1, 4, A], BF16)
            for r in range(4):
                nc.scalar.copy(va_bf_row[0:1, r, :], va_f32[:])
            va_bc4 = consts.tile([128, 4, A], BF16)
            nc.gpsimd.partition_broadcast(
                va_bc4[:].rearrange("p r a -> p (r a)"),
                va_bf_row[:].rearrange("p r a -> p (r a)"),
            )

            # va_b broadcast (bias for exp)
            vab_sb = setup.tile([1, 1], F32)
            nc.sync.dma_start(vab_sb[:], vab_d[:])
            vab_bc = consts.tile([128, 1], F32)
            nc.gpsimd.partition_broadcast(vab_bc[:], vab_sb[:])

            # bias_sum = Wa_b + Ua_b  [1, A]
            wab_sb = setup.tile([1, A], F32, tag="bias")
            nc.sync.dma_start(wab_sb[:], wab_d.ap())
            uab_sb = setup.tile([1, A], F32, tag="bias")
            nc.sync.dma_start(uab_sb[:], uab_d.ap())
            bias_bf = setup.tile([1, A], BF16, tag="biasbf")
            nc.vector.tensor_add(bias_bf[:], wab_sb[:], uab_sb[:])

            # attn2[b, a] = q[b] @ Ua^T + Ua_b + Wa_b   -> [BL, A] psum
            ones_row_bf = setup.tile([1, 128], BF16, tag="onesrow")
            nc.gpsimd.memset(ones_row_bf[:], 1.0)
            a2_ps = psum_a2_pool.tile([BL, A], F32)
            for hc in range(NH_CH):
                nc.tensor.matmul(
                    a2_ps[:],
                    qT[:, hc, :],
                    uaT[:, hc, :],
                    start=(hc == 0),
                    stop=False,
                )
            nc.tensor.matmul(
                a2_ps[:], ones_row_bf[0:1, 0:BL], bias_bf[:], start=False, stop=True
            )
            a2_sb = setup.tile([BL, A], F32)
            nc.scalar.copy(a2_sb[:], a2_ps[:])
            # move the 8 rows to partition 0; a2pk row 0 = attn2[b], rows 1.. = 0
            a2_flat = setup.tile([1, BL, A], F32)
            nc.sync.dma_start(a2_flat[0:1, :, :], a2_sb[:, :])
            a2pk = consts.tile([128, BL, A], BF16)
            nc.vector.memset(a2pk[:], 0.0)
            nc.scalar.copy(a2pk[0:1, :, :], a2_flat[:])

            # ---------------- persistent accumulators ----------------
            w_all = persist.tile([128, NT, BL], F32)
            ctx_ps = [
                psum_acc_pool.tile([128, C], F32, tag=f"ctx{i}", name=f"ctx_ps{i}")
                for i in range(2)
            ]
            z_ps = psum_acc_pool.tile([BL, 1], F32)
            for i in range(2):
                nc.vector.memset(ctx_ps[i][:], 0.0)

            # ---------------- main loop over p-tiles ----------------
            pre = {}
            for t in range(2):
                kN = keysN_pool.tile([PT, BL, C], BF16, name=f"keysN_pre{t}")
                nc.sync.dma_start(kN[:], keysN_d[t])
                kT = keysT_pool.tile([128, BL, NC_CH, PT], FP8, name=f"keysT_pre{t}")
                nc.sync.dma_start(kT[:], keysT_d[t])
                pre[t] = (kN, kT)
            for t in range(NT):
                if t in pre:
                    keysN, keysT = pre.pop(t)
                else:
                    keysN = keysN_pool.tile([PT, BL, C], BF16, name="keysN", tag="keysN_pre0")
                    nc.sync.dma_start(keysN[:], keysN_d[t])
                    keysT = keysT_pool.tile([128, BL, NC_CH, PT], FP8, name="keysT", tag="keysT_pre0")
                    nc.sync.dma_start(keysT[:], keysT_d[t])

                mm1 = [
                    psum_mm1_pool.tile([PT, 4, A], F32, tag="mm1", name="mm1ps")
                    for _ in range(2)
                ]
                for b in range(BL):
                    half, bi = divmod(b, 4)
                    out_ps = mm1[half][:, bi, :]
                    for q in range(2):
                        nc.tensor.matmul(
                            out_ps,
                            keysT[:, b, 2 * q : 2 * q + 2, :],
                            waT[:, 2 * q : 2 * q + 2, :],
                            start=(bi % 2 == 0 and q == 0),
                            stop=False,
                            perf_mode=PM.DoubleRow,
                        )
                    # attn2 add: K=128 stationary with single 1-row, rhs row0=attn2
                    nc.tensor.matmul(
                        out_ps,
                        a2pad[:],
                        a2pk[:, b, :],
                        start=False,
                        stop=(bi % 2 == 1),
                    )
                for half in range(2):
                    t_bf = tanh_pool.tile([PT, 4, A], BF16, tag="tanh")
                    nc.scalar.activation(t_bf[:], mm1[half][:], AF.Tanh)
                    prod = junk_pool.tile([PT, 4, A], BF16, tag="jnk")
                    nc.vector.tensor_mul(prod[:], t_bf[:], va_bc4[:])
                    scores_h = small_pool.tile([128, 4], F32, tag="scores")
                    nc.vector.tensor_reduce(
                        scores_h[:], prod[:], axis=mybir.AxisListType.X, op=ALU.add
                    )
                    # exp(s + va_b) for this half, keep f32 for output
                    nc.scalar.activation(
                        w_all[:, t, 4 * half : 4 * half + 4],
                        scores_h[:],
                        AF.Exp,
                        bias=vab_bc[:],
                    )
                    exp_bf = small_pool.tile([128, 4], BF16, tag="expbf")
                    nc.scalar.copy(exp_bf[:], w_all[:, t, 4 * half : 4 * half + 4])
                    # context: ctx[b] += exp_b^T @ keysN[b]  (one [1,512] psum row per b)
                    for bi in range(4):
                        b = half * 4 + bi
                        nc.tensor.matmul(
                            ctx_ps[half][32 * bi : 32 * bi + 1, :],
                            exp_bf[:, bi : bi + 1],
                            keysN[:, b, :],
                            start=(t == 0),
                            stop=(t == NT - 1),
                            tile_position=(0, 32 * bi),
                        )
                # Z[b] += sum_p exp   (f32, reads w_all directly)
                nc.tensor.matmul(
                    z_ps[:],
                    w_all[:, t, :],
                    ones_col_f32[:],
                    start=(t == 0),
                    stop=(t == NT - 1),
                )

            # ---------------- finalize ----------------
            z_sb = setup.tile([BL, 1], F32, tag="zsb")
            nc.scalar.copy(z_sb[:], z_ps[:])
            rz = setup.tile([BL, 1], F32, tag="rz")
            nc.vector.reciprocal(rz[:], z_sb[:])

            # rz broadcast for the weights normalize
            rz_row = setup.tile([1, BL], F32, tag="rzrow")
            nc.sync.dma_start(rz_row[0:1, :], rz[:, :])
            rz_bc = setup.tile([128, BL], F32, tag="rzbc")
            nc.gpsimd.partition_broadcast(rz_bc[:], rz_row[:])

            # context rows: psum [32*bi] rows -> sbuf -> gather to [BL, C]
            ctx_sb = setup.tile([128, 2, C], F32, tag="ctxsb")
            for half in range(2):
                nc.scalar.copy(ctx_sb[:, half, :], ctx_ps[half][:])
            ctx_f = setup.tile([BL, C], F32, tag="ctxf")
            for half in range(2):
                nc.sync.dma_start(
                    ctx_f[4 * half : 4 * half + 4, :],
                    ctx_sb[0:128:32, half, :],
                )
            nc.vector.tensor_scalar_mul(ctx_f[:], ctx_f[:], rz[:])
            nc.sync.dma_start(ctx_d[:], ctx_f[:])

            # weights: normalize + store in 4 interleaved chunks
            w_view = w_d.ap().rearrange("(t p) b -> t p b", p=PT)
            CH = NT // 4
            for g in range(4):
                for t in range(g * CH, (g + 1) * CH):
                    nc.vector.tensor_mul(
                        w_all[:, t, :], w_all[:, t, :], rz_bc[:]
                    )
                nc.sync.dma_start(
                    w_view[g * CH : (g + 1) * CH].rearrange("t p b -> p t b"),
                    w_all[:, g * CH : (g + 1) * CH, :],
                )


    nc.compile()
    return nc


_NC_CACHE = None


def _get_nc():
    global _NC_CACHE
    if _NC_CACHE is None:
        _NC_CACHE = build_nc()
    return _NC_CACHE


def make_in_maps(inputs):
    """Host-side prep: keys in natural (bf16) and transposed (fp8) tile layouts."""
    keys = np.asarray(inputs["keys"])
    keysN = keys.astype(ml_dtypes.bfloat16).reshape(NT, PT, B, C)
    keysT = np.ascontiguousarray(
        keys.astype(ml_dtypes.float8_e4m3)
        .reshape(NT, PT, B, NC_CH, 128)
        .transpose(0, 4, 2, 3, 1)
    )
    # waT[cc, mc, a] = Wa_w[a, mc*128+cc]
    waT = np.ascontiguousarray(
        np.asarray(inputs["Wa_w"])
        .astype(ml_dtypes.float8_e4m3)
        .reshape(A, NC_CH, 128)
        .transpose(2, 1, 0)
    )
    uaT = np.ascontiguousarray(
        np.asarray(inputs["Ua_w"])
        .astype(ml_dtypes.bfloat16)
        .reshape(A, NH_CH, 128)
        .transpose(2, 1, 0)
    )
    q = np.asarray(inputs["queries"])  # [1, B, H]
    qT_full = (
        q[0].astype(ml_dtypes.bfloat16).reshape(B, NH_CH, 128).transpose(2, 1, 0)
    )  # [hh, hc, b]
    rep = ("Wa_b", "Ua_b", "va_w", "va_b")
    in_maps = []
    for m in range(NCORES):
        sl = slice(m * BL, (m + 1) * BL)
        im = {
            "keysN": np.ascontiguousarray(keysN[:, :, sl, :]),
            "keysT": np.ascontiguousarray(keysT[:, :, sl, :, :]),
            "waT_h": waT,
            "uaT_h": uaT,
            "qT_h": np.ascontiguousarray(qT_full[:, :, sl]),
        }
        for k in rep:
            im[k] = np.asarray(inputs[k])
        in_maps.append(im)
    return in_maps


def kernel(**inputs):
    nc = _get_nc()
    in_maps = make_in_maps(inputs)
    res = bass_utils.run_bass_kernel_spmd(nc, in_maps, core_ids=list(range(NCORES)))
    ctx = np.zeros((1, B, C), np.float32)
    w = np.zeros((P, B, 1), np.float32)
    for m in range(NCORES):
        sl = slice(m * BL, (m + 1) * BL)
        ctx[0, sl, :] = res.results[m]["ctx_out"]
        w[:, sl, 0] = res.results[m]["w_out"]
    return ctx, w


# revision 21
# speedup vs baseline: 1.0378x; 1.0378x over previous
"""Bahdanau-style attention kernel for Trainium2, data-parallel over batch B on 8 NeuronCores.

Reference computation (per batch b):
  attn1[p,a] = sum_c keys[p,b,c] * Wa_w[a,c] + Wa_b[a]
  attn2[a]   = sum_h queries[0,b,h] * Ua_w[a,h] + Ua_b[a]
  scores[p]  = sum_a tanh(attn1[p,a] + attn2[a]) * va_w[0,a] + va_b[0]
  weights    = softmax(scores over p)
  context[c] = sum_p weights[p] * keys[p,b,c]

Scores are tiny (|s| < ~4), so softmax is computed unnormalized: exp(s) is
accumulated into Z and context_unnorm in a single pass over keys, then
normalized at the end.

The PE contracts over the partition axis, so attn1 (contraction over c) needs
keys with c on partitions while the context accumulation (contraction over p)
needs p on partitions.  Both layouts are prepared host-side: keysT in fp8
(feeds only the scores path, where quantization error averages out over the
C=512 contraction) and keysN in bf16 (feeds the context accumulation).  The
device reads 1.5 MB per 128-row tile and does no transposes or casts of keys.
attn1 runs as fp8 DoubleRow matmuls (2 MACs/cell/cycle).  The small weight
operands (WaT, UaT, qT) are also laid out host-side.
"""

import sys

sys.path.insert(0, "/opt/trn_rl_repo")

import ml_dtypes
import numpy as np

from concourse import bacc, bass, mybir, tile
from concourse import bass_utils

P, B, C, H, A = 4096, 64, 512, 512, 256
NCORES = 8
BL = B // NCORES  # 8 batches per core
PT = 128  # rows per p-tile
NT = P // PT  # 32 p-tiles
NC_CH = C // 128  # 4 contraction chunks of 128
NH_CH = H // 128
F32 = mybir.dt.float32
BF16 = mybir.dt.bfloat16
FP8 = mybir.dt.float8e4
AF = mybir.ActivationFunctionType
ALU = mybir.AluOpType
PM = mybir.MatmulPerfMode


def build_nc():
    nc = bacc.Bacc("TRN2", target_bir_lowering=False, debug=False)

    # host-prepared key layouts
    # keysN[t, p, b, c]      = keys[t*128+p, b, c]            bf16 (p on partitions)
    # keysT[t, cc, b, mc, p] = keys[t*128+p, b, mc*128+cc]    fp8  (c-chunk on partitions)
    keysN_d = nc.dram_tensor("keysN", [NT, PT, BL, C], BF16, kind="ExternalInput")
    keysT_d = nc.dram_tensor(
        "keysT", [NT, 128, BL, NC_CH, PT], FP8, kind="ExternalInput"
    )
    # host-prepared weight layouts
    # waT[cc, mc, a] = Wa_w[a, mc*128+cc] fp8 ; uaT likewise bf16 ; qT[hh, hc, b] bf16
    waT_d = nc.dram_tensor("waT_h", [128, NC_CH, A], FP8, kind="ExternalInput")
    uaT_d = nc.dram_tensor("uaT_h", [128, NH_CH, A], BF16, kind="ExternalInput")
    qT_d = nc.dram_tensor("qT_h", [128, NH_CH, BL], BF16, kind="ExternalInput")
    wab_d = nc.dram_tensor("Wa_b", [A], F32, kind="ExternalInput")
    uab_d = nc.dram_tensor("Ua_b", [A], F32, kind="ExternalInput")
    vaw_d = nc.dram_tensor("va_w", [1, A], F32, kind="ExternalInput")
    vab_d = nc.dram_tensor("va_b", [1], F32, kind="ExternalInput")
    ctx_d = nc.dram_tensor("ctx_out", [BL, C], F32, kind="ExternalOutput")
    w_d = nc.dram_tensor("w_out", [P, BL], F32, kind="ExternalOutput")

    with tile.TileContext(nc) as tc:
        with (
            tc.tile_pool(name="consts", bufs=1) as consts,
            tc.tile_pool(name="setup", bufs=2) as setup,
            tc.tile_pool(name="persist", bufs=1) as persist,
            tc.tile_pool(name="keysN", bufs=5) as keysN_pool,
            tc.tile_pool(name="keysT", bufs=5) as keysT_pool,
            tc.tile_pool(name="tanh", bufs=4) as tanh_pool,
            tc.tile_pool(name="junk", bufs=4) as junk_pool,
            tc.tile_pool(name="small", bufs=6) as small_pool,
            tc.tile_pool(name="psum_a2", bufs=1, space="PSUM") as psum_a2_pool,
            tc.tile_pool(name="psum_mm1", bufs=2, space="PSUM") as psum_mm1_pool,
            tc.tile_pool(name="psum_acc", bufs=1, space="PSUM") as psum_acc_pool,
        ):
            # ---------------- constants / small weights ----------------
            waT = consts.tile([128, NC_CH, A], FP8)
            nc.sync.dma_start(waT[:], waT_d[:])
            uaT = consts.tile([128, NH_CH, A], BF16)
            nc.sync.dma_start(uaT[:], uaT_d[:])
            qT = consts.tile([128, NH_CH, BL], BF16)
            nc.sync.dma_start(qT[:], qT_d[:])

            ones_col_bf = consts.tile([128, 1], BF16)
            nc.vector.memset(ones_col_bf[:], 1.0)
            # padded-ones stationary for the attn2 add: row 0 = 1, rows 1.. = 0
            a2pad = consts.tile([128, PT], BF16)
            nc.vector.memset(a2pad[:], 0.0)
            nc.vector.memset(a2pad[0:1, :], 1.0)

            # va broadcast to all partitions, repeated 4x (for the scores reduction)
            va_f32 = setup.tile([1, A], F32)
            nc.sync.dma_start(va_f32[:], vaw_d[:])
            va_bf_row = setup.tile([1, 4, A], BF16)
            for r in range(4):
                nc.scalar.copy(va_bf_row[0:1, r, :], va_f32[:])
            va_bc4 = consts.tile([128, 4, A], BF16)
            nc.gpsimd.partition_broadcast(
                va_bc4[:].rearrange("p r a -> p (r a)"),
                va_bf_row[:].rearrange("p r a -> p (r a)"),
            )

            # va_b broadcast (bias for exp)
            vab_sb = setup.tile([1, 1], F32)
            nc.sync.dma_start(vab_sb[:], vab_d[:])
            vab_bc = consts.tile([128, 1], F32)
            nc.gpsimd.partition_broadcast(vab_bc[:], vab_sb[:])

            # bias_sum = Wa_b + Ua_b  [1, A]
            wab_sb = setup.tile([1, A], F32, tag="bias")
            nc.sync.dma_start(wab_sb[:], wab_d.ap())
            uab_sb = setup.tile([1, A], F32, tag="bias")
            nc.sync.dma_start(uab_sb[:], uab_d.ap())
            bias_bf = setup.tile([1, A], BF16, tag="biasbf")
            nc.vector.tensor_add(bias_bf[:], wab_sb[:], uab_sb[:])

            # attn2[b, a] = q[b] @ Ua^T + Ua_b + Wa_b   -> [BL, A] psum
            ones_row_bf = setup.tile([1, 128], BF16, tag="onesrow")
            nc.gpsimd.memset(ones_row_bf[:], 1.0)
            a2_ps = psum_a2_pool.tile([BL, A], F32)
            for hc in range(NH_CH):
                nc.tensor.matmul(
                    a2_ps[:],
                    qT[:, hc, :],
                    uaT[:, hc, :],
                    start=(hc == 0),
                    stop=False,
                )
            nc.tensor.matmul(
                a2_ps[:], ones_row_bf[0:1, 0:BL], bias_bf[:], start=False, stop=True
            )
            a2_sb = setup.tile([BL, A], F32)
            nc.scalar.copy(a2_sb[:], a2_ps[:])
            # move the 8 rows to partition 0; a2pk row 0 = attn2[b], rows 1.. = 0
            a2_flat = setup.tile([1, BL, A], F32)
            nc.sync.dma_start(a2_flat[0:1, :, :], a2_sb[:, :])
            a2pk = consts.tile([128, BL, A], BF16)
            nc.vector.memset(a2pk[:], 0.0)
            nc.scalar.copy(a2pk[0:1, :, :], a2_flat[:])

            # ---------------- persistent accumulators ----------------
            w_all = persist.tile([128, NT, BL], BF16)
            ctx_ps = [
                psum_acc_pool.tile([128, C], F32, tag=f"ctx{i}", name=f"ctx_ps{i}")
                for i in range(2)
            ]
            z_ps = psum_acc_pool.tile([BL, 1], F32)
            for i in range(2):
                nc.vector.memset(ctx_ps[i][:], 0.0)

            # ---------------- main loop over p-tiles ----------------
            pre = {}
            for t in range(2):
                kN = keysN_pool.tile([PT, BL, C], BF16, name=f"keysN_pre{t}")
                nc.sync.dma_start(kN[:], keysN_d[t])
                kT = keysT_pool.tile([128, BL, NC_CH, PT], FP8, name=f"keysT_pre{t}")
                nc.sync.dma_start(kT[:], keysT_d[t])
                pre[t] = (kN, kT)
            for t in range(NT):
                if t in pre:
                    keysN, keysT = pre.pop(t)
                else:
                    keysN = keysN_pool.tile([PT, BL, C], BF16, name="keysN", tag="keysN_pre0")
                    nc.sync.dma_start(keysN[:], keysN_d[t])
                    keysT = keysT_pool.tile([128, BL, NC_CH, PT], FP8, name="keysT", tag="keysT_pre0")
                    nc.sync.dma_start(keysT[:], keysT_d[t])

                mm1 = [
                    psum_mm1_pool.tile([PT, 4, A], F32, tag="mm1", name="mm1ps")
                    for _ in range(2)
                ]
                for b in range(BL):
                    half, bi = divmod(b, 4)
                    out_ps = mm1[half][:, bi, :]
                    for q in range(2):
                        nc.tensor.matmul(
                            out_ps,
                            keysT[:, b, 2 * q : 2 * q + 2, :],
                            waT[:, 2 * q : 2 * q + 2, :],
                            start=(bi % 2 == 0 and q == 0),
                            stop=False,
                            perf_mode=PM.DoubleRow,
                        )
                    # attn2 add: K=128 stationary with single 1-row, rhs row0=attn2
                    nc.tensor.matmul(
                        out_ps,
                        a2pad[:],
                        a2pk[:, b, :],
                        start=False,
                        stop=(bi % 2 == 1),
                    )
                for half in range(2):
                    t_bf = tanh_pool.tile([PT, 4, A], BF16, tag="tanh")
                    nc.scalar.activation(t_bf[:], mm1[half][:], AF.Tanh)
                    prod = junk_pool.tile([PT, 4, A], BF16, tag="jnk")
                    nc.vector.tensor_mul(prod[:], t_bf[:], va_bc4[:])
                    scores_h = small_pool.tile([128, 4], F32, tag="scores")
                    nc.vector.tensor_reduce(
                        scores_h[:], prod[:], axis=mybir.AxisListType.X, op=ALU.add
                    )
                    # exp(s + va_b) for this half, bf16, straight into w_all
                    nc.scalar.activation(
                        w_all[:, t, 4 * half : 4 * half + 4],
                        scores_h[:],
                        AF.Exp,
                        bias=vab_bc[:],
                    )
                    # context: ctx[b] += exp_b^T @ keysN[b]  (one [1,512] psum row per b)
                    for bi in range(4):
                        b = half * 4 + bi
                        nc.tensor.matmul(
                            ctx_ps[half][32 * bi : 32 * bi + 1, :],
                            w_all[:, t, b : b + 1],
                            keysN[:, b, :],
                            start=(t == 0),
                            stop=(t == NT - 1),
                            tile_position=(0, 32 * bi),
                        )
                # Z[b] += sum_p exp   (f32, reads w_all directly)
                nc.tensor.matmul(
                    z_ps[:],
                    w_all[:, t, :],
                    ones_col_bf[:],
                    start=(t == 0),
                    stop=(t == NT - 1),
                )

            # ---------------- finalize ----------------
            z_sb = setup.tile([BL, 1], F32, tag="zsb")
            nc.scalar.copy(z_sb[:], z_ps[:])
            rz = setup.tile([BL, 1], F32, tag="rz")
            nc.vector.reciprocal(rz[:], z_sb[:])

            # rz broadcast for the weights normalize
            rz_row = setup.tile([1, BL], F32, tag="rzrow")
            nc.sync.dma_start(rz_row[0:1, :], rz[:, :])
            rz_bc = setup.tile([128, BL], F32, tag="rzbc")
            nc.gpsimd.partition_broadcast(rz_bc[:], rz_row[:])

            # context rows: psum [32*bi] rows -> sbuf -> gather to [BL, C]
            ctx_sb = setup.tile([128, 2, C], F32, tag="ctxsb")
            for half in range(2):
                nc.scalar.copy(ctx_sb[:, half, :], ctx_ps[half][:])
            ctx_f = setup.tile([BL, C], F32, tag="ctxf")
            for half in range(2):
                nc.sync.dma_start(
                    ctx_f[4 * half : 4 * half + 4, :],
                    ctx_sb[0:128:32, half, :],
                )
            nc.vector.tensor_scalar_mul(ctx_f[:], ctx_f[:], rz[:])
            nc.sync.dma_start(ctx_d[:], ctx_f[:])

            # weights: normalize + store in 4 interleaved chunks
            w_view = w_d.ap().rearrange("(t p) b -> t p b", p=PT)
            CH = NT // 4
            for g in range(4):
                w_norm = small_pool.tile([128, CH, BL], F32, tag="wnorm")
                for t in range(CH):
                    nc.vector.tensor_mul(
                        w_norm[:, t, :], w_all[:, g * CH + t, :], rz_bc[:]
                    )
                nc.sync.dma_start(
                    w_view[g * CH : (g + 1) * CH].rearrange("t p b -> p t b"),
                    w_norm[:],
                )


    nc.compile()
    return nc


_NC_CACHE = None


def _get_nc():
    global _NC_CACHE
    if _NC_CACHE is None:
        _NC_CACHE = build_nc()
    return _NC_CACHE


def make_in_maps(inputs):
    """Host-side prep: keys in natural (bf16) and transposed (fp8) tile layouts."""
    keys = np.asarray(inputs["keys"])
    keysN = keys.astype(ml_dtypes.bfloat16).reshape(NT, PT, B, C)
    keysT = np.ascontiguousarray(
        keys.astype(ml_dtypes.float8_e4m3)
        .reshape(NT, PT, B, NC_CH, 128)
        .transpose(0, 4, 2, 3, 1)
    )
    # waT[cc, mc, a] = Wa_w[a, mc*128+cc]
    waT = np.ascontiguousarray(
        np.asarray(inputs["Wa_w"])
        .astype(ml_dtypes.float8_e4m3)
        .reshape(A, NC_CH, 128)
        .transpose(2, 1, 0)
    )
    uaT = np.ascontiguousarray(
        np.asarray(inputs["Ua_w"])
        .astype(ml_dtypes.bfloat16)
        .reshape(A, NH_CH, 128)
        .transpose(2, 1, 0)
    )
    q = np.asarray(inputs["queries"])  # [1, B, H]
    qT_full = (
        q[0].astype(ml_dtypes.bfloat16).reshape(B, NH_CH, 128).transpose(2, 1, 0)
    )  # [hh, hc, b]
    rep = ("Wa_b", "Ua_b", "va_w", "va_b")
    in_maps = []
    for m in range(NCORES):
        sl = slice(m * BL, (m + 1) * BL)
        im = {
            "keysN": np.ascontiguousarray(keysN[:, :, sl, :]),
            "keysT": np.ascontiguousarray(keysT[:, :, sl, :, :]),
            "waT_h": waT,
            "uaT_h": uaT,
            "qT_h": np.ascontiguousarray(qT_full[:, :, sl]),
        }
        for k in rep:
            im[k] = np.asarray(inputs[k])
        in_maps.append(im)
    return in_maps


def kernel(**inputs):
    nc = _get_nc()
    in_maps = make_in_maps(inputs)
    res = bass_utils.run_bass_kernel_spmd(nc, in_maps, core_ids=list(range(NCORES)))
    ctx = np.zeros((1, B, C), np.float32)
    w = np.zeros((P, B, 1), np.float32)
    for m in range(NCORES):
        sl = slice(m * BL, (m + 1) * BL)
        ctx[0, sl, :] = res.results[m]["ctx_out"]
        w[:, sl, 0] = res.results[m]["w_out"]
    return ctx, w
